# revision 1
# baseline (speedup 1.0000x reference)
"""4-layer GAT on 8 Trainium2 NeuronCores (v2).

Sharding: destination nodes across the 8 cores (2500 dst rows each); GAT
weights replicated; per-layer AllGather of the projected node features
packed with fp32 alpha_src aux columns into one uint16 row table.

Per layer, per core:
  P1: h_aug_shard = x_shard @ [W | W@a_src | W@a_dst]  (bf16 matmuls, fp32
      PSUM).  Staging packs [per-head channels | 1.0] + alpha_src (fp32) into
      the table row; alpha_dst stays local (hi/lo bf16 pair in SBUF).
  P2: AllGather the table -> full 20000-row DRAM table.
  P3: per 128-edge block (edges CSR-grouped by 128-dst tile, host-side):
      dma_gather of source rows; one-hot S01 = (iota == dst_rel) built on DVE;
      alpha_dst expanded per-edge with PE (transpose(S01) @ ad_local);
      w = exp(leaky_relu(as+ad)); messages scaled by w (DVE/ACT split);
      scatter-matmul acc[dst] += S01^T @ [w*G | w] on the PE, the w-column
      accumulating the softmax denominator.  (The segment softmax is
      shift-invariant, so exp without max-subtraction is exact.)
  P4: out = acc/denom (+ELU for layers 0-2).
Layer 3 is reformulated as aggregate-then-project: the table carries x3
itself, aggregation runs in 1024-dim space, and W3 is applied afterwards
(valid since all messages share W3).
"""

import numpy as np
import ml_dtypes

import concourse.bass as bass
import concourse.bacc as bacc
import concourse.tile as tile
from concourse import mybir, bass_utils

N = 20000
E = 320000
NCORE = 8
NSH = N // NCORE  # 2500 dst rows per core
OUT = 64
NEG = 0.2
NODE_PAD = 2560
NTILE = NODE_PAD // 128
ROW = 1152  # u16 cols per table row (all layers)

AFT = mybir.ActivationFunctionType
ALU = mybir.AluOpType
BF16 = mybir.dt.bfloat16
F32 = mybir.dt.float32
U16 = mybir.dt.uint16
I16 = mybir.dt.int16

# L0-2 row: head h channels at [h*257, h*257+256], ones col at h*257+256;
#           alpha_src (4xf32) at u16 cols 1028..1036.
# L3  row: x3 at [0,1024), ones col 1024, alpha_src (1xf32) at u16 1026..1028.
LAYERS = [
    dict(din=128, KC=1, H=4, C=256, AUX0=1028),
    dict(din=1024, KC=8, H=4, C=256, AUX0=1028),
    dict(din=1024, KC=8, H=4, C=256, AUX0=1028),
    dict(din=1024, KC=8, H=1, C=1024, AUX0=1026),  # payload = x3
]
SENTINEL = 300.0


def _wrap_idx(ids: np.ndarray) -> np.ndarray:
    n = len(ids)
    assert n % 16 == 0
    grp = ids.reshape(n // 16, 16).T.astype(np.int16)
    return np.tile(grp, (8, 1)).copy()


def preprocess_edges(edge_index: np.ndarray):
    src = np.concatenate([edge_index[0], np.arange(N, dtype=edge_index.dtype)])
    dst = np.concatenate([edge_index[1], np.arange(N, dtype=edge_index.dtype)])

    cores = []
    for c in range(NCORE):
        lo, hi = c * NSH, (c + 1) * NSH
        m = (dst >= lo) & (dst < hi)
        es, ed = src[m], dst[m] - lo
        order = np.argsort(ed, kind="stable")
        es, ed = es[order], ed[order]
        tiles = []
        for t in range(NTILE):
            tm = (ed >= t * 128) & (ed < (t + 1) * 128)
            tiles.append((es[tm], ed[tm] - t * 128))
        cores.append(tiles)

    Bt = []
    for t in range(NTILE):
        mx = max(len(cores[c][t][0]) for c in range(NCORE))
        Bt.append(max(1, -(-mx // 128)))
    total = sum(Bt)
    Bt[-1] += (-total) % 8
    nblk = sum(Bt)

    per_core = []
    for c in range(NCORE):
        src_ids = np.zeros(nblk * 128, np.int16)
        dst_rel = np.full(nblk * 128, SENTINEL, np.float32)
        b0 = 0
        for t in range(NTILE):
            es, er = cores[c][t]
            k = len(es)
            src_ids[b0 * 128 : b0 * 128 + k] = es.astype(np.int16)
            dst_rel[b0 * 128 : b0 * 128 + k] = er.astype(np.float32)
            b0 += Bt[t]
        per_core.append(
            dict(
                srcw=_wrap_idx(src_ids),
                dstrel=dst_rel.reshape(nblk, 128).T.copy(),
            )
        )
    return per_core, Bt


def prep_weights(inp: dict):
    ws = {}
    for i in range(4):
        W = np.asarray(inp[f"W{i}"], np.float32)
        a_s = np.asarray(inp[f"a_src{i}"], np.float32)
        a_d = np.asarray(inp[f"a_dst{i}"], np.float32)
        H, C = a_s.shape
        Wh = W.reshape(W.shape[0], H, C)
        Ws = (Wh * a_s[None]).sum(-1)
        Wd = (Wh * a_d[None]).sum(-1)
        if i < 3:
            ws[f"Waug{i}"] = np.concatenate([W, Ws, Wd], axis=1).astype(
                ml_dtypes.bfloat16
            )
        else:
            ws["Waug3"] = np.concatenate([Ws, Wd], axis=1).astype(ml_dtypes.bfloat16)
            ws["W3p"] = W.astype(ml_dtypes.bfloat16)
    return ws


def build_program(Bt: list[int]):
    nblk = sum(Bt)
    niw = nblk * 8
    nc = bacc.Bacc("TRN2", target_bir_lowering=False, debug=False, num_devices=NCORE)

    xT0 = nc.dram_tensor("xT0", [128, NODE_PAD], BF16, kind="ExternalInput").ap()
    wts = []
    for i in range(4):
        shape = [LAYERS[i]["din"], LAYERS[i]["H"] * 256 + 2 * LAYERS[i]["H"]] \
            if i < 3 else [1024, 2]
        wts.append(
            nc.dram_tensor(f"Waug{i}", shape, BF16, kind="ExternalInput").ap()
        )
    w3p_d = nc.dram_tensor("W3p", [1024, OUT], BF16, kind="ExternalInput").ap()
    srcw_d = nc.dram_tensor("srcw", [128, niw], I16, kind="ExternalInput").ap()
    dstrel_d = nc.dram_tensor("dstrel", [128, nblk], F32, kind="ExternalInput").ap()
    riota_d = nc.dram_tensor("riota", [128, 128], F32, kind="ExternalInput").ap()
    ident_d = nc.dram_tensor("identb", [128, 128], BF16, kind="ExternalInput").ap()
    out_d = nc.dram_tensor("out", [NSH, OUT], F32, kind="ExternalOutput").ap()

    blocks = []
    for t in range(NTILE):
        for j in range(Bt[t]):
            blocks.append((t, j == 0, j == Bt[t] - 1))

    with tile.TileContext(nc) as tc:
        with (
            tc.tile_pool(name="dram", bufs=1, space="DRAM") as dram,
            tc.tile_pool(name="ctrl", bufs=1) as ctrl,
        ):
            ag_in = [dram.tile([NSH, ROW], U16, name=f"agin{i}") for i in range(4)]
            hfull = [
                dram.tile([N, ROW], U16, addr_space="Shared", name=f"hfull{i}")
                for i in range(4)
            ]
            xnext = [
                dram.tile([NODE_PAD, 1024], BF16, name=f"xnext{i}") for i in range(3)
            ]
            aggd = dram.tile([NODE_PAD, 1024], BF16, name="aggd")

            srcw = ctrl.tile([128, niw], I16)
            dstrel = ctrl.tile([128, nblk], F32)
            riota = ctrl.tile([128, 128], F32)
            identb = ctrl.tile([128, 128], BF16)
            nc.sync.dma_start(out=srcw[:], in_=srcw_d[:])
            nc.sync.dma_start(out=dstrel[:], in_=dstrel_d[:])
            nc.sync.dma_start(out=riota[:], in_=riota_d[:])
            nc.sync.dma_start(out=identb[:], in_=ident_d[:])
            # zero the node-pad rows of the inter-layer buffers: they feed
            # matmuls (alpha_dst aux) that contract over partitions, so
            # uninitialized DRAM there would poison real outputs.
            zpad = ctrl.tile([NODE_PAD - NSH, 1024], BF16)
            nc.vector.memset(zpad[:], 0.0)
            for i in range(3):
                nc.sync.dma_start(out=xnext[i][NSH:NODE_PAD, :], in_=zpad[:])
            adbuf = [
                ctrl.tile([128, NTILE, 2 * LAYERS[i]["H"]], BF16, name=f"adbuf{i}")
                for i in range(4)
            ]

            for li in range(4):
                L = LAYERS[li]
                if li < 3:
                    emit_p1(nc, tc, li, L, xT0, wts[li], xnext, ag_in[li],
                            adbuf[li])
                else:
                    emit_p1_l3(nc, tc, wts[3], xnext[2], ag_in[3], adbuf[3])
                nc.gpsimd.collective_compute(
                    "AllGather",
                    ALU.bypass,
                    replica_groups=[list(range(NCORE))],
                    ins=[ag_in[li].opt()],
                    outs=[hfull[li].opt()],
                )
                emit_p3(nc, tc, li, L, hfull[li], srcw, dstrel, riota, identb,
                        adbuf[li], blocks, nblk, xnext, aggd)
            emit_post(nc, tc, aggd, w3p_d, out_d)
    nc.compile()
    return nc


def emit_p1(nc, tc, li, L, xT0, wt_d, xnext, ag_in, adbuf):
    """h_aug shard matmul + table staging for layers 0-2."""
    H, C, KC, AUX0 = L["H"], L["C"], L["KC"], L["AUX0"]
    HC = H * C
    NW = HC + 2 * H
    CP1 = C + 1
    with (
        tc.tile_pool(name=f"p1w{li}", bufs=1) as wp,
        tc.tile_pool(name=f"p1x{li}", bufs=1) as xp,
        tc.tile_pool(name=f"p1s{li}", bufs=3) as sp,
        tc.tile_pool(name=f"p1p{li}", bufs=2, space="PSUM") as pp,
    ):
        wt = wp.tile([128, KC, NW], BF16)
        for k in range(KC):
            nc.sync.dma_start(out=wt[:, k, :], in_=wt_d[k * 128 : (k + 1) * 128, :])
        xt = xp.tile([128, KC, NODE_PAD], BF16)
        if li == 0:
            nc.sync.dma_start(out=xt[:, 0, :], in_=xT0[:])
        else:
            for k in range(KC):
                nc.sync.dma_start(
                    out=xt[:, k, :],
                    in_=xnext[li - 1][:, k * 128 : (k + 1) * 128],
                    transpose=True,
                )
        for m in range(NTILE):
            hps = pp.tile([128, NW], F32, space="PSUM", tag="hps")
            nsplits = [(0, 512), (512, 1024), (1024, NW)]
            for k in range(KC):
                lhsT = xt[:, k, m * 128 : (m + 1) * 128]
                for (n0, n1) in nsplits:
                    nc.tensor.matmul(
                        out=hps[:, n0:n1], lhsT=lhsT, rhs=wt[:, k, n0:n1],
                        start=(k == 0), stop=(k == KC - 1),
                    )
            st = sp.tile([128, ROW], U16, tag="stage")
            st_bf = st[:].bitcast(BF16)
            st_f32 = st[:].bitcast(F32)
            for h in range(H):
                nc.vector.tensor_copy(
                    st_bf[:, h * CP1 : h * CP1 + C], hps[:, h * C : (h + 1) * C]
                )
                nc.vector.memset(st_bf[:, h * CP1 + C : h * CP1 + C + 1], 1.0)
            nc.vector.tensor_copy(
                st_f32[:, AUX0 // 2 : AUX0 // 2 + H], hps[:, HC : HC + H]
            )
            nc.vector.tensor_copy(adbuf[:, m, 0:H], hps[:, HC + H : HC + 2 * H])
            nc.vector.tensor_tensor(
                out=adbuf[:, m, H : 2 * H],
                in0=hps[:, HC + H : HC + 2 * H],
                in1=adbuf[:, m, 0:H],
                op=ALU.subtract,
            )
            r0 = m * 128
            rows = min(128, NSH - r0)
            if rows > 0:
                nc.sync.dma_start(out=ag_in[r0 : r0 + rows, :], in_=st[:rows, :])


def emit_p1_l3(nc, tc, wt_d, xnext2, ag_in, adbuf):
    """Layer-3 table: x3 passthrough + alpha aux (Ws3|Wd3 matmul)."""
    AUX0 = LAYERS[3]["AUX0"]
    with (
        tc.tile_pool(name="p1w3", bufs=1) as wp,
        tc.tile_pool(name="p1x3", bufs=1) as xp,
        tc.tile_pool(name="p1s3", bufs=3) as sp,
        tc.tile_pool(name="p1p3", bufs=2, space="PSUM") as pp,
    ):
        wt = wp.tile([128, 8, 2], BF16)
        for k in range(8):
            nc.sync.dma_start(out=wt[:, k, :], in_=wt_d[k * 128 : (k + 1) * 128, :])
        xt = xp.tile([128, 8, NODE_PAD], BF16)
        for k in range(8):
            nc.sync.dma_start(
                out=xt[:, k, :], in_=xnext2[:, k * 128 : (k + 1) * 128],
                transpose=True,
            )
        for m in range(NTILE):
            aux = pp.tile([128, 2], F32, space="PSUM", tag="aux3")
            for k in range(8):
                nc.tensor.matmul(
                    out=aux[:], lhsT=xt[:, k, m * 128 : (m + 1) * 128],
                    rhs=wt[:, k, :], start=(k == 0), stop=(k == 7),
                )
            st = sp.tile([128, ROW], U16, tag="stage3")
            st_bf = st[:].bitcast(BF16)
            st_f32 = st[:].bitcast(F32)
            nc.sync.dma_start(
                out=st_bf[:, 0:1024], in_=xnext2[m * 128 : (m + 1) * 128, :]
            )
            nc.vector.memset(st_bf[:, 1024:1025], 1.0)
            nc.vector.tensor_copy(st_f32[:, AUX0 // 2 : AUX0 // 2 + 1], aux[:, 0:1])
            nc.vector.tensor_copy(adbuf[:, m, 0:1], aux[:, 1:2])
            nc.vector.tensor_tensor(
                out=adbuf[:, m, 1:2], in0=aux[:, 1:2], in1=adbuf[:, m, 0:1],
                op=ALU.subtract,
            )
            r0 = m * 128
            rows = min(128, NSH - r0)
            if rows > 0:
                nc.sync.dma_start(out=ag_in[r0 : r0 + rows, :], in_=st[:rows, :])


def emit_p3(nc, tc, li, L, hfull, srcw, dstrel, riota, identb, adbuf,
            blocks, nblk, xnext, aggd):
    H, C, AUX0 = L["H"], L["C"], L["AUX0"]
    CP1 = C + 1
    as0 = AUX0 // 2
    if li < 3:
        jobs = [(h * CP1, h * CP1 + CP1, h) for h in range(H)]
    else:
        jobs = [(0, 512, 0), (512, 1024, 0), (1024, 1025, 0)]
    with (
        tc.tile_pool(name=f"p3g{li}", bufs=2) as gp,
        tc.tile_pool(name=f"p3s{li}", bufs=18) as sp,
        tc.tile_pool(name=f"p3t{li}", bufs=4) as tp,
        tc.tile_pool(name=f"p3z{li}", bufs=2) as zp,
        tc.tile_pool(name=f"p3gs{li}", bufs=8) as gsp,
        tc.tile_pool(name=f"p3e{li}", bufs=2) as ep,
        tc.tile_pool(name=f"p3acc{li}", bufs=1, space="PSUM") as accp,
        tc.tile_pool(name=f"p3tp{li}", bufs=2, space="PSUM") as trp,
        tc.tile_pool(name=f"p3ad{li}", bufs=2, space="PSUM") as adp,
    ):
        def emit_chunk(ci):
            g = gp.tile([128, 8, ROW], U16, tag="g1")
            nc.gpsimd.dma_gather(
                g[:], hfull[:], srcw[:, ci * 64 : ci * 64 + 64],
                1024, 1024, ROW,
            )
            ps_ad = adp.tile([128, 8, 2 * H], F32, space="PSUM", tag="psad")
            s01s = []
            for bj in range(8):
                b = ci * 8 + bj
                t = blocks[b][0]
                s01 = sp.tile([128, 128], BF16, tag="s01")
                nc.vector.tensor_scalar(
                    out=s01[:], in0=riota[:], scalar1=dstrel[:, b : b + 1],
                    scalar2=None, op0=ALU.is_equal,
                )
                s01t_ps = trp.tile([128, 128], BF16, space="PSUM", tag="s01t_ps")
                nc.tensor.transpose(out=s01t_ps[:], in_=s01[:], identity=identb[:])
                s01t = tp.tile([128, 128], BF16, tag="s01t")
                nc.vector.tensor_copy(s01t[:], s01t_ps[:])
                nc.tensor.matmul(
                    out=ps_ad[:, bj, :], lhsT=s01t[:], rhs=adbuf[:, t, :],
                    start=True, stop=True,
                )
                s01s.append(s01)
            gf = g[:].bitcast(F32)
            z = zp.tile([128, 8, H], F32, tag="z")
            nc.vector.tensor_tensor(
                out=z[:], in0=gf[:, :, as0 : as0 + H], in1=ps_ad[:, :, 0:H],
                op=ALU.add,
            )
            nc.vector.tensor_tensor(
                out=z[:], in0=z[:], in1=ps_ad[:, :, H : 2 * H], op=ALU.add
            )
            z2 = zp.tile([128, 8, H], F32, tag="z2")
            nc.vector.tensor_scalar_mul(z2[:], z[:], NEG)
            zm = zp.tile([128, 8, H], F32, tag="zm")
            nc.vector.tensor_tensor(out=zm[:], in0=z[:], in1=z2[:], op=ALU.max)
            w = zp.tile([128, 8, H], F32, tag="w")
            nc.scalar.activation(w[:], zm[:], AFT.Exp)
            return g, s01s, w

        acc = None
        g = s01s = w = g_bf = None
        for b, (t, first, last) in enumerate(blocks):
            ci, bj = b // 8, b % 8
            if bj == 0:
                g, s01s, w = emit_chunk(ci)
                g_bf = g[:].bitcast(BF16)
            if first:
                acc = accp.tile(
                    [128, H, 512] if li < 3 else [128, 1536],
                    F32, space="PSUM", tag="acc",
                )
            for ji, (c0, c1, h) in enumerate(jobs):
                width = c1 - c0
                gs = gsp.tile([128, 512 if li == 3 else CP1], BF16, tag="gs")
                if ji % 2 == 0:
                    nc.vector.tensor_scalar(
                        out=gs[:, 0:width], in0=g_bf[:, bj, c0:c1],
                        scalar1=w[:, bj, h : h + 1], scalar2=None, op0=ALU.mult,
                    )
                else:
                    nc.scalar.mul(gs[:, 0:width], g_bf[:, bj, c0:c1],
                                  w[:, bj, h : h + 1])
                o = acc[:, h, 0:CP1] if li < 3 else acc[:, c0:c1]
                nc.tensor.matmul(
                    out=o, lhsT=s01s[bj][:], rhs=gs[:, 0:width],
                    start=first, stop=last,
                )
            if last:
                emit_epilogue(nc, tc, li, L, t, acc, ep, xnext, aggd)


def emit_epilogue(nc, tc, li, L, t, acc, ep, xnext, aggd):
    H, C = L["H"], L["C"]
    r0 = t * 128
    rows = min(128, NSH - r0)
    if rows <= 0:
        return
    den = ep.tile([128, H], F32, tag="den")
    if li < 3:
        nc.vector.tensor_copy(den[:], acc[:, :, C])
    else:
        nc.vector.tensor_copy(den[:], acc[:, 1024:1025])
    rec = ep.tile([128, H], F32, tag="rec")
    nc.vector.reciprocal(rec[:], den[:])
    if li < 3:
        xstage = ep.tile([128, 1024], BF16, tag="xst")
        for h in range(H):
            tmp = ep.tile([128, C], F32, tag="tmp")
            nc.vector.tensor_scalar(
                out=tmp[:], in0=acc[:, h, 0:C],
                scalar1=rec[:, h : h + 1], scalar2=None, op0=ALU.mult,
            )
            mn = ep.tile([128, C], F32, tag="mn")
            nc.vector.tensor_scalar_min(mn[:], tmp[:], 0.0)
            ex = ep.tile([128, C], F32, tag="ex")
            nc.scalar.activation(ex[:], mn[:], AFT.Exp)
            mx = ep.tile([128, C], F32, tag="mx")
            nc.vector.tensor_scalar_max(mx[:], tmp[:], 0.0)
            sm = ep.tile([128, C], F32, tag="sm")
            nc.vector.tensor_tensor(out=sm[:], in0=mx[:], in1=ex[:], op=ALU.add)
            nc.vector.tensor_scalar(
                out=xstage[:, h * C : (h + 1) * C], in0=sm[:],
                scalar1=-1.0, scalar2=None, op0=ALU.add,
            )
        nc.sync.dma_start(out=xnext[li][r0 : r0 + rows, :], in_=xstage[:rows, :])
    else:
        astage = ep.tile([128, 1024], BF16, tag="ast")
        nc.vector.tensor_scalar(
            out=astage[:], in0=acc[:, 0:1024],
            scalar1=rec[:, 0:1], scalar2=None, op0=ALU.mult,
        )
        nc.sync.dma_start(out=aggd[r0 : r0 + rows, :], in_=astage[:rows, :])


def emit_post(nc, tc, aggd, w3p_d, out_d):
    """out = (agg/den) @ W3."""
    with (
        tc.tile_pool(name="pow", bufs=1) as wp,
        tc.tile_pool(name="pox", bufs=1) as xp,
        tc.tile_pool(name="pos", bufs=3) as sp,
        tc.tile_pool(name="pop", bufs=2, space="PSUM") as pp,
    ):
        wt = wp.tile([128, 8, OUT], BF16)
        for k in range(8):
            nc.sync.dma_start(out=wt[:, k, :], in_=w3p_d[k * 128 : (k + 1) * 128, :])
        xt = xp.tile([128, 8, NODE_PAD], BF16)
        for k in range(8):
            nc.sync.dma_start(
                out=xt[:, k, :], in_=aggd[:, k * 128 : (k + 1) * 128],
                transpose=True,
            )
        for m in range(NTILE):
            ps = pp.tile([128, OUT], F32, space="PSUM", tag="ops")
            for k in range(8):
                nc.tensor.matmul(
                    out=ps[:], lhsT=xt[:, k, m * 128 : (m + 1) * 128],
                    rhs=wt[:, k, :], start=(k == 0), stop=(k == 7),
                )
            ost = sp.tile([128, OUT], F32, tag="ost")
            nc.vector.tensor_copy(ost[:], ps[:])
            r0 = m * 128
            rows = min(128, NSH - r0)
            if rows > 0:
                nc.sync.dma_start(out=out_d[r0 : r0 + rows, :], in_=ost[:rows, :])


# ------------------------------------------------------------------
# host-side driver with persistent compiled executor
# ------------------------------------------------------------------
_CACHE: dict = {}


def _get_executor(Bt_key, Bt):
    if Bt_key in _CACHE:
        return _CACHE[Bt_key]
    import jax
    from jax.sharding import Mesh, PartitionSpec
    from jax.experimental.shard_map import shard_map
    from concourse import bass2jax

    nc = build_program(Bt)
    bass2jax.install_neuronx_cc_hook()
    partition_name = nc.partition_id_tensor.name if nc.partition_id_tensor else None
    in_names, out_names, out_avals, zero_shapes = [], [], [], []
    for alloc in nc.m.functions[0].allocations:
        if not isinstance(alloc, mybir.MemoryLocationSet):
            continue
        name = alloc.memorylocations[0].name
        if alloc.kind == "ExternalInput":
            if name != partition_name:
                in_names.append(name)
        elif alloc.kind == "ExternalOutput":
            shape = tuple(alloc.tensor_shape)
            dtype = mybir.dt.np(alloc.dtype)
            out_avals.append(jax.core.ShapedArray(shape, dtype))
            out_names.append(name)
            zero_shapes.append((shape, dtype))
    n_params = len(in_names)
    in_names_all = list(in_names) + out_names
    if partition_name is not None:
        in_names_all.append(partition_name)

    def _body(*args):
        operands = list(args)
        if partition_name is not None:
            operands.append(bass2jax.partition_id_tensor())
        outs = bass2jax._bass_exec_p.bind(
            *operands,
            out_avals=tuple(out_avals),
            in_names=tuple(in_names_all),
            out_names=tuple(out_names),
            lowering_input_output_aliases=(),
            sim_require_finite=False,
            sim_require_nnan=False,
            nc=nc,
        )
        return tuple(outs)

    devices = jax.devices()[:NCORE]
    mesh = Mesh(np.asarray(devices), ("core",))
    n_outs = len(out_names)
    in_specs = (PartitionSpec("core"),) * (n_params + n_outs)
    out_specs = (PartitionSpec("core"),) * n_outs
    fn = jax.jit(
        shard_map(_body, mesh=mesh, in_specs=in_specs, out_specs=out_specs,
                  check_rep=False),
        keep_unused=True,
    )
    ex = dict(fn=fn, in_names=in_names, out_names=out_names,
              zero_shapes=zero_shapes, nc=nc, body=_body, mesh=mesh,
              n_params=n_params, n_outs=n_outs)
    _CACHE[Bt_key] = ex
    return ex


def _prepare_inputs(inputs):
    x = np.asarray(inputs["x"], np.float32)
    edge_index = np.asarray(inputs["edge_index"])
    per_core, Bt = preprocess_edges(edge_index)
    ws = prep_weights(inputs)
    riota = np.tile(np.arange(128, dtype=np.float32), (128, 1)).copy()
    identb = np.eye(128, dtype=ml_dtypes.bfloat16)
    xb = x.astype(ml_dtypes.bfloat16)
    in_maps = []
    for c in range(NCORE):
        xsh = np.zeros((128, NODE_PAD), ml_dtypes.bfloat16)
        xsh[:, :NSH] = xb[c * NSH : (c + 1) * NSH, :].T
        m = dict(
            xT0=xsh,
            srcw=per_core[c]["srcw"],
            dstrel=per_core[c]["dstrel"],
            riota=riota,
            identb=identb,
        )
        m.update(ws)
        in_maps.append(m)
    return in_maps, Bt


def kernel(**inputs) -> np.ndarray:
    import jax

    in_maps, Bt = _prepare_inputs(inputs)
    ex = _get_executor(tuple(Bt), Bt)
    args = []
    for name in ex["in_names"]:
        args.append(np.concatenate([m[name] for m in in_maps], axis=0))
    for shape, dtype in ex["zero_shapes"]:
        args.append(np.zeros((NCORE * shape[0], *shape[1:]), dtype))
    outs = ex["fn"](*args)
    jax.block_until_ready(outs)
    oidx = ex["out_names"].index("out")
    full = np.asarray(outs[oidx])
    return full.astype(np.float32)


def measure_exec_time(inputs, reps: int = 10) -> float:
    """Estimate device exec time (ns) per run via repeat-dispatch slope."""
    import time
    import jax
    from jax.sharding import NamedSharding, PartitionSpec

    in_maps, Bt = _prepare_inputs(inputs)
    ex = _get_executor(tuple(Bt), Bt)
    args = [
        np.concatenate([m[name] for m in in_maps], axis=0)
        for name in ex["in_names"]
    ]
    args += [
        np.zeros((NCORE * s[0], *s[1:]), d) for (s, d) in ex["zero_shapes"]
    ]
    sh = NamedSharding(ex["mesh"], PartitionSpec("core"))
    dargs = [jax.device_put(a, sh) for a in args]
    o = ex["fn"](*dargs)
    jax.block_until_ready(o)

    def run(R):
        t0 = time.perf_counter()
        outs = [ex["fn"](*dargs) for _ in range(R)]
        jax.block_until_ready(outs)
        return time.perf_counter() - t0

    t1 = min(run(1) for _ in range(5))
    tR = min(run(reps) for _ in range(3))
    per_iter_s = (tR - t1) / (reps - 1)
    print(f"[timing] t1={t1*1e3:.1f}ms  t{reps}={tR*1e3:.1f}ms  "
          f"slope={per_iter_s*1e3:.2f}ms/iter")
    return per_iter_s * 1e9

